# revision 27
# baseline (speedup 1.0000x reference)
"""MultiHeadSelfAttention TRN2 kernel — head-tensor-parallel over 8 NeuronCores.

Reference semantics (note the quirk: softmax over the QUERY axis):
    Q = x @ Wq[h].T + bq[h]            [B,S,D] per head
    K = x @ Wk[h].T + bk[h]
    V = x @ Wv[h].T + bv[h]
    scores[s,t] = (Q[s]·K[t]) / sqrt(D)
    attn = softmax over s (query axis)  -> attn[s,t] = exp(sc[s,t]) / sum_s' exp(sc[s',t])
    Z[s] = sum_t attn[s,t] V[t]
    out = concat_heads(Z) @ Wo.T + bo

Sharding: head h -> core h. Each core computes its head's partial output
projection out_h = Z_h @ Wo[:, h*D:(h+1)*D].T ; host sums the 8 partials
(the all-reduce after W_o, done on host during the gather) ; bo is folded
into core 0's partial.

Layout strategy (everything transposed so the quirky softmax normalization
axis 's' lands on the free dimension):
    xT   [d, s]   QT = WqT.T @ xT   [e, s]
    KT   [e, t],  V [t, e]
    scoresT[t, s] = KT.T @ QT  -> exp with ACT accum_out => denom[t] for free
    V'[t,:] = V[t,:] * (C / denom[t])   (stored fp8)
    ZT[e, s] = V'.T @ PT   via fp8e4 DoubleRow matmuls (two t-blocks of
               contraction per instruction, 2x PE rate)
    outT[o, s] = WoHT.T @ ZT            (Wo pre-scaled by 1/C on host)
Projections and scores run in float32r (full PE rate).

fp8 scaling: PT = exp(score - ln PBIAS) keeps values below the TRN e4m3
max of 240 (256 encodes Inf).  V' = (C*V) / dn with dn = denom/PBIAS, so
ZT = C * sum_t attn*V and 1/C is folded into Wo host-side.

Schedule (per batch): QK projections (Q drains via ACT Identity+bias, K
via DVE) -> scores superblock g0 (8 t-blocks) -> norm -> scores g1 ->
norm.  The exp stream on ACT (1.1us per [128,1024] slice) is slower than
the PE's score production (0.85us), so ACT-independent PE work is
interleaved between score slices as "fillers": the V projection chunks of
this batch, the g1 ZT quarters + output projection of the PREVIOUS batch
(cross-batch software pipelining), and the g0 ZT quarters of this batch
during the g1 scores.  Fillers use the 2 "z" PSUM banks; score slices own
the 6 "acc" banks, so filler PSUM pressure never blocks the ACT pipeline.

PSUM budget: acc [128,1024] x3 bufs = 6 banks, z0/z1 [128,512] = 2 banks.
"""

import numpy as np

import concourse.bass as bass
import concourse.mybir as mybir
import concourse.tile as tile
from concourse import bacc
from concourse.bass_utils import run_bass_kernel_spmd

B, S, D, H = 4, 2048, 256, 8
N_CORES = 8
P = 128          # partitions
NDB = D // P     # 2 d-blocks (contraction blocks for projections)
NTB = S // P     # 16 key/t blocks
SC = 512         # matmul moving-dim chunk == z psum tile width
NSC = S // SC    # 4 s chunks
SH = 1024        # s-half (scores psum + output DMA granularity)
NSH = S // SH    # 2 s halves
G = 8            # t-blocks per superblock
NSUP = NTB // G  # 2 superblocks

f32 = mybir.dt.float32
f32r = mybir.dt.float32r
f8 = mybir.dt.float8e4
DR = mybir.MatmulPerfMode.DoubleRow
EXP = mybir.ActivationFunctionType.Exp
IDN = mybir.ActivationFunctionType.Identity

# fp8 scaling constants (see module docstring)
PBIAS = 8.0
VC = 1024.0


def _build():
    nc = bacc.Bacc(target_bir_lowering=False)

    xT = nc.dram_tensor("xT", [B, D, S], f32, kind="ExternalInput")
    wqT = nc.dram_tensor("wqT", [D, D], f32, kind="ExternalInput")  # [d,e] = (Wq/sqrt(D)).T
    wkT = nc.dram_tensor("wkT", [D, D], f32, kind="ExternalInput")  # [d,e]
    wvT = nc.dram_tensor("wvT", [D, D], f32, kind="ExternalInput")  # [d,e] * VC
    woT = nc.dram_tensor("woT", [D, D], f32, kind="ExternalInput")  # [e,o] / VC
    bqc = nc.dram_tensor("bqc", [D, 1], f32, kind="ExternalInput")
    bkc = nc.dram_tensor("bkc", [D, 1], f32, kind="ExternalInput")
    bvb = nc.dram_tensor("bvb", [P, 2 * D], f32, kind="ExternalInput")  # bv*VC x2, bcast
    boc = nc.dram_tensor("boc", [D, 1], f32, kind="ExternalInput")  # bo (core0) / zeros
    ebc = nc.dram_tensor("ebc", [P, 1], f32, kind="ExternalInput")  # -ln(PBIAS) bcast
    outT = nc.dram_tensor("outT", [B, D, S], f32, kind="ExternalOutput")

    with tile.TileContext(nc) as tc:
        with (
            tc.tile_pool(name="const", bufs=1) as cpool,
            tc.tile_pool(name="big", bufs=1) as xpool,
            tc.tile_pool(name="xtp", bufs=2) as xtpool,
            tc.tile_pool(name="ktp", bufs=2) as ktpool,
            tc.tile_pool(name="ztp", bufs=2) as zpool,
            tc.tile_pool(name="pt", bufs=2) as ppool,
            tc.tile_pool(name="small", bufs=2) as spool,
            tc.tile_pool(name="outp", bufs=4) as opool,
            tc.tile_pool(name="ps_a", bufs=3, space="PSUM") as psa,
            tc.tile_pool(name="ps_z", bufs=1, space="PSUM") as psz,
        ):
            # ---- constants (once) ----
            # wq on the (otherwise idle at startup) scalar queue, x on sync —
            # they land in parallel and gate the very first matmuls; the rest
            # go on the gpsimd queue.
            wq_t = cpool.tile([P, NDB, D], f32r, tag="wq")
            wk_t = cpool.tile([P, NDB, D], f32r, tag="wk")
            wv_t = cpool.tile([P, NDB, D], f32r, tag="wv")
            wo_t = cpool.tile([P, NDB, D], f32r, tag="wo")
            nc.scalar.dma_start(
                out=wq_t[:], in_=wqT.rearrange("(n p) e -> p n e", p=P).bitcast(f32r)
            )
            for w_t, w_d in ((wk_t, wkT), (wv_t, wvT), (wo_t, woT)):
                nc.gpsimd.dma_start(
                    out=w_t[:], in_=w_d.rearrange("(n p) e -> p n e", p=P).bitcast(f32r)
                )
            bq_t = cpool.tile([P, NDB, 1], f32, tag="bq")
            bk_t = cpool.tile([P, NDB, 1], f32, tag="bk")
            bo_t = cpool.tile([P, NDB, 1], f32, tag="bo")
            bvb_t = cpool.tile([P, 2 * D], f32, tag="bvb")
            for b_t, b_d in ((bq_t, bqc), (bk_t, bkc)):
                nc.gpsimd.dma_start(
                    out=b_t[:], in_=b_d.rearrange("(n p) o -> p n o", p=P)
                )
            nc.gpsimd.dma_start(out=bvb_t[:], in_=bvb[:])
            nc.gpsimd.dma_start(
                out=bo_t[:], in_=boc.rearrange("(n p) o -> p n o", p=P)
            )
            eb_t = cpool.tile([P, 1], f32, tag="ebc")
            nc.gpsimd.dma_start(out=eb_t[:], in_=ebc[:])

            zctr = [0]

            def ztag():
                zctr[0] += 1
                return f"z{zctr[0] % 2}"

            def emit_v_chunk(xt, v_all, c):
                """V projection for t-blocks 2c, 2c+1 -> v_all (+bias)."""
                psv = psz.tile([P, 2 * D], f32, tag=ztag(), name="psv")
                for k in range(2):
                    tb = 2 * c + k
                    for db in range(NDB):
                        nc.tensor.matmul(
                            psv[:, bass.ts(k, D)],
                            xt[:, db, bass.ts(tb, P)],
                            wv_t[:, db, :],
                            start=(db == 0),
                            stop=(db == NDB - 1),
                        )
                nc.vector.tensor_add(
                    v_all[:, bass.ds(2 * c, 2), :],
                    psv[:].rearrange("p (g e) -> p g e", g=2),
                    bvb_t[:].rearrange("p (g e) -> p g e", g=2),
                )

            def emit_zt_q(zt, pt, vp, g, sq, eh):
                """One ZT quarter (sq, eh) of superblock g — fp8 DoubleRow."""
                psz_t = psz.tile([P, SC], f32, tag=ztag(), name="psz_t")
                ssl = bass.ts(sq, SC)
                for m in range(G // 2):
                    nc.tensor.matmul(
                        psz_t[:],
                        vp[:, 2 * m : 2 * m + 2, bass.ts(eh, P)],
                        pt[:, 2 * m : 2 * m + 2, ssl],
                        start=(m == 0),
                        stop=(m == G // 2 - 1),
                        perf_mode=DR,
                    )
                zsl = zt[:, eh, ssl]
                if g == 0:
                    nc.vector.tensor_copy(zsl, psz_t[:])
                else:
                    nc.vector.tensor_add(zsl, zsl, psz_t[:])

            def emit_op_item(b, zt, ob, sh, sc):
                """One 512-wide chunk of the output projection of batch b."""
                osb = opool.tile([P, SC], f32, tag="osb", name="osb")
                pso = psz.tile([P, SC], f32, tag=ztag(), name="pso")
                ssl = bass.ds(sh * SH + sc * SC, SC)
                for eh in range(NDB):
                    nc.tensor.matmul(
                        pso[:],
                        wo_t[:, eh, bass.ts(ob, P)],
                        zt[:, eh, ssl],
                        start=(eh == 0),
                        stop=(eh == NDB - 1),
                    )
                nc.vector.tensor_scalar_add(osb[:], pso[:], bo_t[:, ob, :])
                eng = nc.sync if (ob + sh + sc) % 2 == 0 else nc.gpsimd
                eng.dma_start(out=outT[b, bass.ts(ob, P), ssl], in_=osb[:])

            def emit_scores_slice(qt, kt, pt, dnp, g, j, sh):
                """scores + biased exp for t-block g*G+j, query half sh."""
                tb = g * G + j
                pssc = psa.tile([P, SH], f32, tag="acc", name="pssc")
                for sc in range(SH // SC):
                    ssl = bass.ds(sh * SH + sc * SC, SC)
                    psl = bass.ts(sc, SC)
                    for eb in range(NDB):
                        nc.tensor.matmul(
                            pssc[:, psl],
                            kt[:, eb, bass.ts(tb, P)],
                            qt[:, eb, ssl],
                            start=(eb == 0),
                            stop=(eb == NDB - 1),
                        )
                nc.scalar.activation(
                    pt[:, j, bass.ts(sh, SH)],
                    pssc[:],
                    EXP,
                    bias=eb_t[:],
                    accum_out=dnp[:, j, sh : sh + 1],
                )

            def emit_norm_half(v_all, vp, dnp, g, h):
                """denominators -> reciprocal -> fp8 V' for half a superblock.
                Emitting the first half early (its exps drain sooner) lets the
                first ZT quarters start before the last exps finish."""
                hg = G // 2
                dn = spool.tile([P, hg], f32, tag=f"dn{h}", name="dn")
                rc = spool.tile([P, hg], f32, tag=f"rc{h}", name="rc")
                jsl = bass.ds(h * hg, hg)
                nc.vector.tensor_add(dn[:], dnp[:, jsl, 0], dnp[:, jsl, 1])
                nc.vector.reciprocal(rc[:], dn[:])
                for j in range(hg):
                    ja = h * hg + j
                    nc.vector.tensor_scalar_mul(
                        vp[:, ja, :], v_all[:, g * G + ja, :], rc[:, j : j + 1]
                    )

            def load_xt(b):
                """Allocate + DMA xt for batch b: s-chunks striped across the
                sync and gpsimd queues so the sh0 half lands in half the time."""
                xt = xtpool.tile([P, NDB, S], f32r, tag="xt", name="xt")
                xT_r = xT[b].rearrange("(n p) s -> p n s", p=P).bitcast(f32r)
                for sq in range(NSC):
                    eng = nc.sync if sq % 2 == 0 else nc.gpsimd
                    eng.dma_start(
                        out=xt[:, :, bass.ts(sq, SC)], in_=xT_r[:, :, bass.ts(sq, SC)]
                    )
                return xt

            def emit_proj_chunk(xt, dst, w, bias, eb, sh, on_act):
                """One [128,1024] psum chunk of a Q/K projection + drain."""
                ps = psa.tile([P, SH], f32, tag="acc", name="ps")
                for sc in range(SH // SC):
                    ssl = bass.ds(sh * SH + sc * SC, SC)
                    psl = bass.ts(sc, SC)
                    for db in range(NDB):
                        nc.tensor.matmul(
                            ps[:, psl],
                            w[:, db, bass.ts(eb, P)],
                            xt[:, db, ssl],
                            start=(db == 0),
                            stop=(db == NDB - 1),
                        )
                dsl = dst[:, eb, bass.ts(sh, SH)]
                if on_act:
                    nc.scalar.activation(dsl, ps[:], IDN, bias=bias[:, eb, :])
                else:
                    nc.vector.tensor_scalar_add(dsl, ps[:], bias[:, eb, :])

            prev = None  # (batch, zt, pt_g1, vp_g1)
            nxt = None   # (xt, kt) of batch b prepared during batch b-1
            for b in range(B):
                if nxt is None:
                    xt = load_xt(b)
                    kt = ktpool.tile([P, NDB, S], f32r, tag="kt", name="kt")
                else:
                    xt, kt = nxt

                # ---- Q projection, sh0 half only (DVE drain — the ACT queue
                # still holds the previous batch's trailing exps).  The sh1
                # chunks become early g0 fillers; scores slice (j, sh) only
                # reads the sh half of qt.  K of this batch was projected
                # during the previous batch's g1 window (b=0: sh0 here). ----
                qt = xpool.tile([P, NDB, S], f32r, tag="qt")
                for eb in range(NDB):
                    emit_proj_chunk(xt, qt, wq_t, bq_t, eb, 0, False)
                if nxt is None:
                    for eb in range(NDB):
                        emit_proj_chunk(xt, kt, wk_t, bk_t, eb, 0, False)

                v_all = xpool.tile([P, NTB, D], f32, tag="v")
                zt = zpool.tile([P, NDB, S], f32r, tag="zt")
                pt0 = ppool.tile([P, G, S], f8, tag="pt", name="pt0")
                vp0 = ppool.tile([P, G, D], f8, tag="vp", name="vp0")
                dnp0 = spool.tile([P, G, NSH], f32, tag="dnp", name="dnp0")
                pt1 = ppool.tile([P, G, S], f8, tag="pt", name="pt1")
                vp1 = ppool.tile([P, G, D], f8, tag="vp", name="vp1")
                dnp1 = spool.tile([P, G, NSH], f32, tag="dnp", name="dnp1")

                # ---- filler items: ~1 ACT-independent PE item between
                # consecutive score slices keeps ACT's exp stream (1.1us per
                # [128,1024] slice vs 0.85us of PE score work) from gating PE.
                Vi = [lambda c=c: emit_v_chunk(xt, v_all, c) for c in range(8)]
                g0q = [
                    lambda sq=sq, eh=eh: emit_zt_q(zt, pt0, vp0, 0, sq, eh)
                    for sq in range(NSC)
                    for eh in range(NDB)
                ]
                # q sh1-half chunks: needed from slice (sh1, j0) = index 8 on;
                # drained on ACT for b>0 (slack there), DVE for b0.
                Pq = [
                    lambda eb=eb, oa=(prev is not None): emit_proj_chunk(
                        xt, qt, wq_t, bq_t, eb, 1, oa
                    )
                    for eb in range(NDB)
                ]
                if prev is not None:
                    pb, pzt, ppt1, pvp1 = prev
                    Qi = [
                        lambda sq=sq, eh=eh: emit_zt_q(pzt, ppt1, pvp1, 1, sq, eh)
                        for sq in range(NSC)
                        for eh in range(NDB)
                    ]
                    Oi = [
                        lambda ob=ob, sh=sh, sc=sc: emit_op_item(pb, pzt, ob, sh, sc)
                        for sh in range(NSH)
                        for ob in range(NDB)
                        for sc in range(2)
                    ]
                    fill0 = [
                        Pq[0], Pq[1], Vi[0], Vi[1],
                        Qi[0], Qi[1], Qi[2], Qi[3],
                        Oi[0], Oi[1], Vi[2], Vi[3],
                        Qi[4], Qi[5], Qi[6], Qi[7],
                        Oi[2], Oi[3],
                    ]
                    fill1 = [Oi[4], Oi[5], Vi[4], Vi[5], Oi[6], Oi[7], Vi[6], Vi[7]]
                else:
                    Pk = [
                        lambda eb=eb: emit_proj_chunk(xt, kt, wk_t, bk_t, eb, 1, False)
                        for eb in range(NDB)
                    ]
                    fill0 = [Pq[0], Pq[1], Pk[0], Pk[1],
                             Vi[0], Vi[1], Vi[2], Vi[3], Vi[4], Vi[5]]
                    fill1 = [Vi[6], Vi[7]]

                def run_scores(pt, dnp, g, fill):
                    fi = 0
                    for sh in range(NSH):
                        for j in range(G):
                            emit_scores_slice(qt, kt, pt, dnp, g, j, sh)
                            if sh == 1 and j == 3:
                                emit_norm_half(v_all, vp0 if g == 0 else vp1, dnp, g, 0)
                            if fi < len(fill):
                                fill[fi]()
                                fi += 1
                    emit_norm_half(v_all, vp0 if g == 0 else vp1, dnp, g, 1)
                    while fi < len(fill):
                        fill[fi]()
                        fi += 1

                run_scores(pt0, dnp0, 0, fill0)

                # g1 tail fillers: next batch's K projection (its xt DMA is
                # emitted here so the transfer queues after the g0-window
                # output DMAs), then this batch's g0 ZT quarters (vp0 lands
                # right after the norm-g0 halves).
                if b + 1 < B:
                    nxt_xt = load_xt(b + 1)
                    nxt_kt = ktpool.tile([P, NDB, S], f32r, tag="kt", name="kt")
                    fill1 += [
                        lambda eb=eb, sh=sh: emit_proj_chunk(
                            nxt_xt, nxt_kt, wk_t, bk_t, eb, sh, False
                        )
                        for sh in range(NSH)
                        for eb in range(NDB)
                    ]
                    nxt = (nxt_xt, nxt_kt)
                else:
                    nxt = None
                fill1 += g0q

                run_scores(pt1, dnp1, 1, fill1)

                prev = (b, zt, pt1, vp1)

            # ---- tail: last batch's g1 ZT quarters + output projection ----
            pb, pzt, ppt1, pvp1 = prev
            tq = [
                lambda sq=sq, eh=eh: emit_zt_q(pzt, ppt1, pvp1, 1, sq, eh)
                for sq in range(NSC)
                for eh in range(NDB)
            ]
            to = [
                lambda ob=ob, sh=sh, sc=sc: emit_op_item(pb, pzt, ob, sh, sc)
                for sh in range(NSH)
                for ob in range(NDB)
                for sc in range(2)
            ]
            for f in (tq[0], tq[1], tq[2], tq[3], to[0], to[1], to[2], to[3],
                      tq[4], tq[5], tq[6], tq[7], to[4], to[5], to[6], to[7]):
                f()

    nc.compile()
    return nc


_NC = None


def _get_nc():
    global _NC
    if _NC is None:
        _NC = _build()
    return _NC


def _make_in_maps(x, Wq, bq, Wk, bk, Wv, bv, Wo, bo):
    x = np.asarray(x, np.float32)
    scale = np.float32(1.0 / np.sqrt(D))
    xT = np.ascontiguousarray(x.transpose(0, 2, 1))
    in_maps = []
    for h in range(H):
        bvh = np.asarray(bv, np.float32)[h]
        m = {
            "xT": xT,
            "wqT": np.ascontiguousarray(np.asarray(Wq, np.float32)[h].T) * scale,
            "wkT": np.ascontiguousarray(np.asarray(Wk, np.float32)[h].T),
            "wvT": np.ascontiguousarray(np.asarray(Wv, np.float32)[h].T) * np.float32(VC),
            "woT": np.ascontiguousarray(np.asarray(Wo, np.float32)[:, h * D : (h + 1) * D].T)
            * np.float32(1.0 / VC),
            "bqc": (np.asarray(bq, np.float32)[h] * scale).reshape(D, 1),
            "bkc": np.asarray(bk, np.float32)[h].reshape(D, 1),
            "bvb": np.ascontiguousarray(
                np.broadcast_to(np.tile(bvh * np.float32(VC), 2), (P, 2 * D))
            ),
            "boc": (
                np.asarray(bo, np.float32) if h == 0 else np.zeros(D, np.float32)
            ).reshape(D, 1),
            "ebc": np.full((P, 1), -np.log(PBIAS), np.float32),
        }
        in_maps.append({k: np.ascontiguousarray(v, np.float32) for k, v in m.items()})
    return in_maps


def kernel(x, Wq, bq, Wk, bk, Wv, bv, Wo, bo, _trace=False, _trace_kwargs=None):
    in_maps = _make_in_maps(x, Wq, bq, Wk, bk, Wv, bv, Wo, bo)
    nc = _get_nc()
    kw = {}
    if _trace:
        kw = dict(trace=True, **(_trace_kwargs or {}))
    br = run_bass_kernel_spmd(nc, in_maps, core_ids=list(range(N_CORES)), **kw)
    acc = np.zeros((B, D, S), np.float32)
    for r in br.results:
        acc += r["outT"]
    out = np.ascontiguousarray(acc.transpose(0, 2, 1))
    if _trace:
        kernel.last_results = br
    return out
